# revision 5
# baseline (speedup 1.0000x reference)
"""Block-sparse attention (nn_BlockSparseAttention) on 8 TRN2 NeuronCores.

Strategy: head-parallel (16 heads / 8 cores = 2 heads per core).
Per core, all in bf16 on the TensorEngine with f32 PSUM accumulation.

Schedule (v3): pipelines the collectives behind compute instead of the
baseline's all-QKV -> all-attention -> serialized AllGather+o_proj tail
(which left the PE idle >100us):
  1. K and V projections chunk-major with both heads interleaved, so
     the single hidden-states DMA queue (2MB per chunk) stays ahead of
     the PE consuming one chunk per ~14us. RoPE is fused into the PSUM
     eviction; V is transposed to natural layout via DMA transpose and
     per-block V sums feed the masked-block correction. Then all Q
     projections, after which the hidden/weights/rope pool is released.
  2. Attention runs chunk by chunk (4 chunks of 512 queries); each
     chunk's AllGather triggers as soon as both heads finish, and
     o_proj for chunk qc-1 is interleaved after attention of chunk qc,
     by which point its AllGather has landed.
  3. The last chunk gathers per-head so o_proj can consume head 0's
     shard while head 1's is in flight (c_order interleave).
Softmax without max-subtraction; reference mask semantics (masked
scores = 0 => prob contribution exp(0) = 1) via the correction
decomposition: exp runs unmasked on the Scalar engine, a bf16 binary
mask zeroes unselected blocks, and masked-block contributions are
restored with two tiny matmuls (V block-sums x complement mask; 64 x
complement count). The mask is pre-expanded to token resolution on the
host and streamed JIT (2MB per head-chunk, double-buffered) so the
mask-multiply is a dense contiguous bf16 op that hits the Vector
engine's 2x packed mode -- the v2 broadcast form ran at 1x and made
DVE the pipeline bottleneck.
Host side: input rearrangement/casting, top-k block mask + token-res
expansion, RoPE tables, final concat+transpose of the 8 output shards.
"""
import sys

if "/opt/trn_rl_repo" not in sys.path:
    sys.path.insert(0, "/opt/trn_rl_repo")

import numpy as np
import ml_dtypes

import concourse.bass as bass
import concourse.tile as tile
import concourse.mybir as mybir
from concourse import bacc
from concourse.bass_utils import run_bass_kernel_spmd
from concourse.masks import make_identity

# problem constants (hardcoded per harness contract)
B, S, HID = 1, 2048, 2048
NH, HD, BS = 16, 128, 64
RATIO = 0.5
THETA = 10000.0
NCORES = 8
HPC = NH // NCORES          # heads per core = 2
P = 128                     # partitions
CH = HID // P               # contraction chunks = 16
KT = S // P                 # key tiles = 16
FB = 512                    # free-dim block (psum bank)
QC = S // FB                # q chunks = 4
NQB = S // BS               # 32 blocks per side
QB_PER_FB = FB // BS        # 8 q-blocks per 512 chunk
KTB = 2                     # key tiles evicted per ACT/DVE op
NP_ = KT // KTB             # score groups per (h, qc) = 8

BF = mybir.dt.bfloat16
F32 = mybir.dt.float32

_CACHE = {}


def _build():
    nc = bacc.Bacc("TRN2", target_bir_lowering=False, debug=False,
                   num_devices=NCORES)

    hT = nc.dram_tensor("hT", [QC, P, CH, FB], BF, kind="ExternalInput").ap()
    wq = nc.dram_tensor("wq", [HPC, P, CH, P], BF, kind="ExternalInput").ap()
    wk = nc.dram_tensor("wk", [HPC, P, CH, P], BF, kind="ExternalInput").ap()
    wv = nc.dram_tensor("wv", [HPC, P, CH, P], BF, kind="ExternalInput").ap()
    wo = nc.dram_tensor("wo", [P, CH, HPC * P], BF, kind="ExternalInput").ap()
    cosT = nc.dram_tensor("cosT", [P, S], F32, kind="ExternalInput").ap()
    sinT = nc.dram_tensor("sinT", [P, S], F32, kind="ExternalInput").ap()  # pre-signed
    binE = nc.dram_tensor("binE", [HPC, QC, P, KT, FB], BF,
                          kind="ExternalInput").ap()
    binN = nc.dram_tensor("binN", [NQB, HPC, NQB], BF, kind="ExternalInput").ap()
    out = nc.dram_tensor("out", [HPC * P, S], F32, kind="ExternalOutput").ap()

    with tile.TileContext(nc) as tc:
        with (
            tc.tile_pool(name="cp", bufs=1) as cp,          # persistent tensors
            tc.tile_pool(name="pp", bufs=1, space="PSUM") as pp,
            tc.tile_pool(name="dp", bufs=1, space="DRAM") as dp,
        ):
            QTr = cp.tile([P, HPC, S], BF, name="QTr")
            KTr = cp.tile([P, HPC, S], BF, name="KTr")
            V_sbs = [cp.tile([P, KT, P], BF, name=f"V_h{h}")
                     for h in range(HPC)]
            corrT_sb = cp.tile([NQB, HPC, P], BF, name="corrT_sb")

            # ---------------- input DMAs (queue assignment matters) --------
            qp = tc.alloc_tile_pool(name="qp", bufs=2)
            # scalar HWDGE: first K weight, then RoPE tables (needed at the
            # first K eviction ~10us in)
            wk_sbs, wv_sbs, wq_sbs = [], [], []
            wk0 = qp.tile([P, CH, P], BF, name="w_sb", tag="w_sb", bufs=6)
            nc.scalar.dma_start(wk0[:], wk[0])
            wk_sbs.append(wk0)
            cos_sb = qp.tile([P, S], F32, name="cos_sb", bufs=1)
            nc.scalar.dma_start(cos_sb[:], cosT[:])
            sin_sb = qp.tile([P, S], F32, name="sin_sb", bufs=1)
            nc.scalar.dma_start(sin_sb[:], sinT[:])
            # gpsimd SWDGE (slow ~9us spinup, fine for later-needed data):
            # remaining K/V weights first, then Q weights / wo / binN
            wk1 = qp.tile([P, CH, P], BF, name="w_sb", tag="w_sb", bufs=6)
            nc.gpsimd.dma_start(wk1[:], wk[1])
            wk_sbs.append(wk1)
            for h in range(HPC):
                wv_sb = qp.tile([P, CH, P], BF, name="w_sb", tag="w_sb", bufs=6)
                nc.gpsimd.dma_start(wv_sb[:], wv[h])
                wv_sbs.append(wv_sb)
            for h in range(HPC):
                wq_sb = qp.tile([P, CH, P], BF, name="w_sb", tag="w_sb", bufs=6)
                nc.gpsimd.dma_start(wq_sb[:], wq[h])
                wq_sbs.append(wq_sb)
            # hidden^T: all 16 subtiles on the sync queue, in consumption
            # order; chunk-major KV (below) consumes 2MB per ~14us while
            # sync delivers 2MB per ~5.6us
            CSUB = 4
            hT_sbs = []
            for qcb in range(QC):
                subs = []
                for cs in range(CSUB):
                    hT_c = qp.tile([P, CH // CSUB, FB], BF,
                                   name=f"hT_c{qcb}_{cs}", bufs=1)
                    nc.sync.dma_start(
                        hT_c[:], hT[qcb, :, cs * (CH // CSUB):
                                     (cs + 1) * (CH // CSUB), :])
                    subs.append(hT_c)
                hT_sbs.append(subs)
            binN_sb = cp.tile([NQB, HPC, NQB], BF, name="binN_sb")
            nc.gpsimd.dma_start(binN_sb[:], binN[:])
            wo_sb = cp.tile([P, CH, HPC * P], BF, name="wo_sb")
            nc.gpsimd.dma_start(wo_sb[:], wo[:])
            ones_sb = cp.tile([P, P], BF, name="ones_sb")
            nc.vector.memset(ones_sb[:], 1.0)
            c64_sb = cp.tile([NQB, P], BF, name="c64_sb")
            nc.vector.memset(c64_sb[:], float(BS))
            ident = cp.tile([P, P], BF, name="ident")
            make_identity(nc, ident[:])

            # ---------------- phase 1: K, V projections -------------------
            def emit_proj_chain(w_sb, qc):
                ps = pp.tile([P, FB], F32, name="ps_acc", tag="ps_acc",
                             bufs=2)
                for c in range(CH):
                    nc.tensor.matmul(
                        ps[:],
                        lhsT=w_sb[:, c, :],
                        rhs=hT_sbs[qc][c // (CH // CSUB)][
                            :, c % (CH // CSUB), :],
                        start=(c == 0),
                        stop=(c == CH - 1),
                    )
                return ps

            def emit_rope_evict(ps, dst, h, qc):
                qsl = slice(qc * FB, (qc + 1) * FB)
                tcos = qp.tile([P, FB], F32, name="tcos", tag="tcos")
                nc.vector.tensor_mul(
                    out=tcos[:], in0=ps[:], in1=cos_sb[:, qsl])
                tsin = qp.tile([P, FB], F32, name="tsin", tag="tsin")
                nc.vector.tensor_mul(
                    out=tsin[0:64, :], in0=ps[64:128, :],
                    in1=sin_sb[0:64, qsl])
                nc.vector.tensor_mul(
                    out=tsin[64:128, :], in0=ps[0:64, :],
                    in1=sin_sb[64:128, qsl])
                nc.vector.tensor_add(
                    out=dst[:, h, qsl], in0=tcos[:], in1=tsin[:])

            vT_sbs = {}
            # chunk-major, heads interleaved: per chunk K(h0), K(h1),
            # V(h0), V(h1) -- so hT chunk qc is consumed over ~14us while
            # the sync queue delivers the next one in ~5.6us
            for qc in range(QC):
                for h in range(HPC):
                    ps = emit_proj_chain(wk_sbs[h], qc)
                    emit_rope_evict(ps, KTr, h, qc)
                for h in range(HPC):
                    ps = emit_proj_chain(wv_sbs[h], qc)
                    vT_c = qp.tile([P, FB], BF, name="vT_c",
                                   tag="vT_c", bufs=8)
                    nc.scalar.copy(out=vT_c[:], in_=ps[:])
                    vT_sbs[(h, qc)] = vT_c
                    eng = nc.scalar if h == 0 else nc.sync
                    for j in range(QC):
                        kt = qc * QC + j
                        eng.dma_start(
                            V_sbs[h][:, kt, :],
                            vT_c[:, j * P:(j + 1) * P],
                            transpose=True,
                        )
            # block-sums of V^T -> [d, kb] -> transpose -> corrT [kb, d]
            for h in range(HPC):
                bsum = qp.tile([P, NQB], BF, name="bsum", tag="bsum")
                with nc.allow_low_precision(
                        reason="block-sum correction term, 64-wide bf16 sum"):
                    for qc in range(QC):
                        nc.vector.tensor_reduce(
                            out=bsum[:, qc * QB_PER_FB:(qc + 1) * QB_PER_FB],
                            in_=vT_sbs[(h, qc)].rearrange(
                                "p (b e) -> p b e", e=BS),
                            axis=mybir.AxisListType.X,
                            op=mybir.AluOpType.add,
                        )
                ps_t = pp.tile([NQB, P], BF, name="ps_t", tag="ps_s", bufs=2)
                with nc.allow_low_precision(
                        reason="block-sum correction term, 64-wide bf16 sum"):
                    nc.tensor.transpose(ps_t[:], bsum[:], ident[:])
                nc.scalar.copy(out=corrT_sb[:, h, :], in_=ps_t[:])

            # ---------------- phase 2: Q projections ----------------------
            for h in range(HPC):
                for qc in range(QC):
                    ps = emit_proj_chain(wq_sbs[h], qc)
                    emit_rope_evict(ps, QTr, h, qc)

            qp.release()

            # ------------- phase 3: attention + pipelined AG + o_proj -----
            wp = tc.alloc_tile_pool(name="wp", bufs=2)
            cc_ins = {qc: dp.tile([HPC * P, FB], BF, name=f"cc_in{qc}")
                      for qc in range(QC - 1)}
            cc_outs = {qc: dp.tile([NCORES * HPC * P, FB], BF,
                                   name=f"cc_out{qc}", addr_space="Shared")
                       for qc in range(QC - 1)}
            # last chunk: per-head AGs so o_proj can overlap the final one
            cc_ins_s = {}
            cc_outs_s = {}
            for qc in (QC - 1,):
                for h in range(HPC):
                    cc_ins_s[(qc, h)] = dp.tile(
                        [P, FB], BF, name=f"cc_ins{qc}_{h}")
                    cc_outs_s[(qc, h)] = dp.tile(
                        [NCORES * P, FB], BF, name=f"cc_outs{qc}_{h}",
                        addr_space="Shared")

            # token-res masks, streamed JIT: h0 tiles on the scalar queue,
            # h1 on gpsimd, double-buffered 2MB apiece
            binE_sbs = {}
            for h in range(HPC):
                for qc in range(QC):
                    mb = wp.tile([P, KT, FB], BF, name=f"binE{h}",
                                 tag=f"binE{h}", bufs=2)
                    eng = nc.scalar if h == 0 else nc.gpsimd
                    eng.dma_start(mb[:], binE[h, qc])
                    binE_sbs[(h, qc)] = mb

            def emit_oproj(qc):
                ag_sb = wp.tile([P, CH, FB], BF, name="ag_sb", tag="ag_sb")
                nc.sync.dma_start(
                    ag_sb[:], cc_outs[qc].rearrange("(c p) s -> p c s", p=P))
                for strip in range(HPC):
                    ssl = slice(strip * P, (strip + 1) * P)
                    ps_w = pp.tile([P, FB], F32, name="ps_w", tag="ps_acc",
                                   bufs=2)
                    for c in range(CH):
                        nc.tensor.matmul(
                            ps_w[:],
                            lhsT=wo_sb[:, c, ssl],
                            rhs=ag_sb[:, c, :],
                            start=(c == 0),
                            stop=(c == CH - 1),
                        )
                    ot = wp.tile([P, FB], F32, name="ot", tag="ot")
                    nc.vector.tensor_copy(out=ot[:], in_=ps_w[:])
                    nc.sync.dma_start(
                        out[strip * P:(strip + 1) * P, qc * FB:(qc + 1) * FB],
                        ot[:],
                    )

            c_order = [c for c in range(CH) if c % HPC == 0] + \
                      [c for c in range(CH) if c % HPC != 0]

            def emit_oproj_split(qc):
                # per-head-gathered chunk: head 0's features consumed first
                # so work overlaps the second AG
                ags = []
                for hh in range(HPC):
                    ag_l = wp.tile([P, NCORES, FB], BF, name="ag_l",
                                   tag="ag_sb")
                    nc.sync.dma_start(
                        ag_l[:],
                        cc_outs_s[(qc, hh)].rearrange("(b p) s -> p b s", p=P))
                    ags.append(ag_l)
                for strip in range(HPC):
                    ssl = slice(strip * P, (strip + 1) * P)
                    ps_w = pp.tile([P, FB], F32, name="ps_w", tag="ps_acc",
                                   bufs=2)
                    for i, c in enumerate(c_order):
                        nc.tensor.matmul(
                            ps_w[:],
                            lhsT=wo_sb[:, c, ssl],
                            rhs=ags[c % HPC][:, c // HPC, :],
                            start=(i == 0),
                            stop=(i == CH - 1),
                        )
                    ot = wp.tile([P, FB], F32, name="ot", tag="ot")
                    nc.vector.tensor_copy(out=ot[:], in_=ps_w[:])
                    nc.sync.dma_start(
                        out[strip * P:(strip + 1) * P, qc * FB:(qc + 1) * FB],
                        ot[:],
                    )

            def emit_attention(h, qc):
                qsl = slice(qc * FB, (qc + 1) * FB)
                qbsl = slice(qc * QB_PER_FB, (qc + 1) * QB_PER_FB)
                mb = binE_sbs[(h, qc)]
                pts = []
                for ktp in range(NP_):
                    ps_s = pp.tile([P, KTB, FB], F32, name="ps_s",
                                   tag="ps_s", bufs=2)
                    for j in range(KTB):
                        kt = KTB * ktp + j
                        nc.tensor.matmul(
                            ps_s[:, j, :],
                            lhsT=KTr[:, h, kt * P:(kt + 1) * P],
                            rhs=QTr[:, h, qsl],
                            start=True, stop=True,
                        )
                    pt = wp.tile([P, KTB, FB], BF, name="probsT",
                                 tag="probsT", bufs=2 * NP_)
                    nc.scalar.activation(
                        out=pt[:], in_=ps_s[:],
                        func=mybir.ActivationFunctionType.Exp)
                    # dense contiguous bf16 mask-mul (2x DVE mode)
                    nc.vector.tensor_mul(
                        out=pt[:],
                        in0=pt[:],
                        in1=mb[:, KTB * ktp:KTB * (ktp + 1), :],
                    )
                    pts.append(pt)

                # PV + masked-block correction
                ps_o = pp.tile([P, FB], F32, name="ps_o", tag="ps_o", bufs=1)
                for kt in range(KT):
                    nc.tensor.matmul(
                        ps_o[:],
                        lhsT=V_sbs[h][:, kt, :],
                        rhs=pts[kt // KTB][:, kt % KTB, :],
                        start=(kt == 0), stop=False,
                    )
                binN_ap = binN_sb[:, h, qbsl]
                nc.tensor.matmul(
                    ps_o[:],
                    lhsT=corrT_sb[:, h, :],
                    rhs=binN_ap[:, :, None].to_broadcast(
                        [NQB, QB_PER_FB, BS]),
                    start=False, stop=True,
                )

                # denominator + count correction
                ps_d = pp.tile([P, FB], F32, name="ps_d", tag="ps_d", bufs=1)
                for kt in range(KT):
                    nc.tensor.matmul(
                        ps_d[:],
                        lhsT=ones_sb[:],
                        rhs=pts[kt // KTB][:, kt % KTB, :],
                        start=(kt == 0), stop=False,
                    )
                nc.tensor.matmul(
                    ps_d[:],
                    lhsT=c64_sb[:],
                    rhs=binN_ap[:, :, None].to_broadcast(
                        [NQB, QB_PER_FB, BS]),
                    start=False, stop=True,
                )
                rden = wp.tile([P, FB], F32, name="rden", tag="rden")
                nc.vector.reciprocal_approx_fast(out=rden[:], in_=ps_d[:])
                at_c = wp.tile([P, FB], BF, name="at_c", tag="at_c", bufs=4)
                nc.vector.tensor_mul(out=at_c[:], in0=ps_o[:], in1=rden[:])

                # deposit + trigger collective
                if qc < QC - 1:
                    nc.sync.dma_start(
                        cc_ins[qc][h * P:(h + 1) * P, :], at_c[:])
                    if h == HPC - 1:
                        nc.gpsimd.collective_compute(
                            "AllGather",
                            mybir.AluOpType.bypass,
                            replica_groups=[list(range(NCORES))],
                            ins=[cc_ins[qc].opt()],
                            outs=[cc_outs[qc].opt()],
                        )
                else:
                    nc.sync.dma_start(cc_ins_s[(qc, h)][:], at_c[:])
                    nc.gpsimd.collective_compute(
                        "AllGather",
                        mybir.AluOpType.bypass,
                        replica_groups=[list(range(NCORES))],
                        ins=[cc_ins_s[(qc, h)].opt()],
                        outs=[cc_outs_s[(qc, h)].opt()],
                    )

            for qc in range(QC):
                for h in range(HPC):
                    emit_attention(h, qc)
                if qc >= 1:
                    emit_oproj(qc - 1)
            emit_oproj_split(QC - 1)
            wp.release()

    nc.compile()
    return nc


def _host_prep(hidden_states, q_w, k_w, v_w, o_w, sparsity_pattern):
    hs = np.asarray(hidden_states, dtype=np.float32).reshape(S, HID)
    qw = np.asarray(q_w, dtype=np.float32)
    kw = np.asarray(k_w, dtype=np.float32)
    vw = np.asarray(v_w, dtype=np.float32)
    ow = np.asarray(o_w, dtype=np.float32)
    sp = np.asarray(sparsity_pattern, dtype=np.float32)

    bf = ml_dtypes.bfloat16

    # hidden^T -> [qcb, p, c, s'] (s-chunk-major so chunk DMAs are contiguous)
    hT = np.ascontiguousarray(
        hs.T.reshape(CH, P, QC, FB).transpose(2, 1, 0, 3)).astype(bf)

    # block mask with per-head top-k threshold
    kk = max(1, int(NH * NQB * NQB * RATIO / NH))
    flat = sp.reshape(NH, -1)
    th = np.partition(flat, -kk, axis=1)[:, -kk]
    bm = (sp > th[:, None, None]).astype(np.float32)  # [NH, 32 qb, 32 kb]

    # RoPE tables in [d, s] layout; sin pre-signed for rotate_half
    inv = 1.0 / (THETA ** (np.arange(0, HD, 2, dtype=np.float32) / HD))
    fr = np.arange(S, dtype=np.float32)[:, None] * inv[None, :]  # [S, 64]
    embT = np.ascontiguousarray(np.concatenate([fr, fr], axis=1).T)  # [128,S]
    cosT = np.cos(embT).astype(np.float32)
    sinT = np.sin(embT).astype(np.float32)
    sinT[:64] *= -1.0

    def w_per_head(w, h, scale=1.0):
        # [HID, 128] -> [p, c, d]
        return np.ascontiguousarray(
            (w[:, h * HD:(h + 1) * HD] * scale)
            .reshape(CH, P, HD).transpose(1, 0, 2))

    def mask_tok(h):
        # bm[h] is [q_block, k_block] -> token-res [keys, q] -> kernel
        # layout [QC, P, KT, FB] (keys on partitions per key-tile)
        m = np.repeat(np.repeat(bm[h].T.astype(bf), BS, axis=0), BS, axis=1)
        return np.ascontiguousarray(
            m.reshape(KT, P, QC, FB).transpose(2, 1, 0, 3))

    qscale = 1.0 / np.sqrt(HD)
    in_maps = []
    for r in range(NCORES):
        heads = [HPC * r + i for i in range(HPC)]
        wq_r = np.stack([w_per_head(qw, h, qscale) for h in heads]).astype(bf)
        wk_r = np.stack([w_per_head(kw, h) for h in heads]).astype(bf)
        wv_r = np.stack([w_per_head(vw, h) for h in heads]).astype(bf)
        wo_r = np.ascontiguousarray(
            ow[:, r * HPC * HD:(r + 1) * HPC * HD]
            .reshape(CH, P, HPC * HD).transpose(1, 0, 2)).astype(bf)
        mE = np.stack([mask_tok(h) for h in heads])  # [HPC, QC, P, KT, FB]
        # complement mask [kb, h, qb] for the masked-block corrections
        mN = np.stack([1.0 - bm[h].T for h in heads], axis=1)  # [32, HPC, 32]
        in_maps.append({
            "hT": hT,
            "wq": wq_r, "wk": wk_r, "wv": wv_r, "wo": wo_r,
            "cosT": cosT, "sinT": sinT,
            "binE": np.ascontiguousarray(mE),
            "binN": np.ascontiguousarray(mN).astype(bf),
        })
    return in_maps


def _run(inputs, trace=False, **kwargs):
    if "nc" not in _CACHE:
        _CACHE["nc"] = _build()
    nc = _CACHE["nc"]
    in_maps = _host_prep(**inputs)
    res = run_bass_kernel_spmd(
        nc, in_maps, core_ids=list(range(NCORES)), trace=trace, **kwargs)
    outT = np.empty((HID, S), dtype=np.float32)
    for r in range(NCORES):
        outT[r * HPC * P:(r + 1) * HPC * P] = \
            np.asarray(res.results[r]["out"], dtype=np.float32)
    full = np.ascontiguousarray(outT.T).reshape(B, S, HID)
    return full, res


def kernel(**inputs):
    full, _ = _run(inputs, trace=False)
    return full


# revision 6
# speedup vs baseline: 1.1783x; 1.1783x over previous
"""Block-sparse attention (nn_BlockSparseAttention) on 8 TRN2 NeuronCores.

Strategy: head-parallel (16 heads / 8 cores = 2 heads per core).
Per core, all in bf16 on the TensorEngine with f32 PSUM accumulation.

Schedule (v4): pipelines the collectives behind compute instead of the
baseline's all-QKV -> all-attention -> serialized AllGather+o_proj tail
(which left the PE idle >100us):
  1. Projections run chunk-major (per 512-token chunk: K, V, Q for both
     heads) so hidden-states chunks stream through a double-buffered
     32KB/partition window on one DMA queue that stays ahead of the PE.
     RoPE is fused into the PSUM eviction; V is transposed to natural
     layout via DMA transpose on the otherwise-idle sync queue (DMA
     transpose burns issuing-engine time, so keeping it off scalar
     stops it blocking PSUM evictions); per-block V sums feed the
     masked-block correction.
  2. Attention runs chunk by chunk; each chunk's AllGather triggers as
     soon as both heads finish. o_proj for chunks 0/1 is deferred until
     after attention chunk 2 so the first AllGather (which absorbs
     cross-core launch skew) has ~50us of slack before the PE needs it.
  3. The last chunk gathers per-head so o_proj can consume head 0's
     shard while head 1's is in flight (c_order interleave).
Softmax without max-subtraction; reference mask semantics (masked
scores = 0 => prob contribution exp(0) = 1) via the correction
decomposition: exp runs unmasked on the Scalar engine, a bf16 binary
mask zeroes unselected blocks, and masked-block contributions are
restored with two tiny matmuls (V block-sums x complement mask; 64 x
complement count). Half the key-tile pairs are pre-summed on Vector so
the denominator ones-matmul pass streams 12 tiles instead of 16,
keeping PE (the attention bottleneck) and DVE balanced.
All attention-phase tiles live in the persistent pool: allocating them
in a post-release pool made their buffers overlap the hidden-states
region, stalling the first exp/mask ops until every projection reader
finished (v3 lost ~15us there).
Host side: input rearrangement/casting, top-k block mask, RoPE tables,
and final concat+transpose of the 8 output shards.
"""
import sys

if "/opt/trn_rl_repo" not in sys.path:
    sys.path.insert(0, "/opt/trn_rl_repo")

import numpy as np
import ml_dtypes

import concourse.bass as bass
import concourse.tile as tile
import concourse.mybir as mybir
from concourse import bacc
from concourse.bass_utils import run_bass_kernel_spmd
from concourse.masks import make_identity

# problem constants (hardcoded per harness contract)
B, S, HID = 1, 2048, 2048
NH, HD, BS = 16, 128, 64
RATIO = 0.5
THETA = 10000.0
NCORES = 8
HPC = NH // NCORES          # heads per core = 2
P = 128                     # partitions
CH = HID // P               # contraction chunks = 16
KT = S // P                 # key tiles = 16
FB = 512                    # free-dim block (psum bank)
QC = S // FB                # q chunks = 4
NQB = S // BS               # 32 blocks per side
QB_PER_FB = FB // BS        # 8 q-blocks per 512 chunk
KTB = 2                     # key tiles evicted per ACT/DVE op
NP_ = KT // KTB             # score groups per (h, qc) = 8
NPAIR = 4                   # key-tile pairs pre-summed on DVE for the denom

BF = mybir.dt.bfloat16
F32 = mybir.dt.float32

_CACHE = {}


def _build():
    nc = bacc.Bacc("TRN2", target_bir_lowering=False, debug=False,
                   num_devices=NCORES)

    hT = nc.dram_tensor("hT", [QC, P, CH, FB], BF, kind="ExternalInput").ap()
    wq = nc.dram_tensor("wq", [HPC, P, CH, P], BF, kind="ExternalInput").ap()
    wk = nc.dram_tensor("wk", [HPC, P, CH, P], BF, kind="ExternalInput").ap()
    wv = nc.dram_tensor("wv", [HPC, P, CH, P], BF, kind="ExternalInput").ap()
    wo = nc.dram_tensor("wo", [P, CH, HPC * P], BF, kind="ExternalInput").ap()
    cosT = nc.dram_tensor("cosT", [P, S], F32, kind="ExternalInput").ap()
    sinT = nc.dram_tensor("sinT", [P, S], F32, kind="ExternalInput").ap()  # pre-signed
    binT = nc.dram_tensor("binT", [P, HPC, KT, NQB], BF, kind="ExternalInput").ap()
    binN = nc.dram_tensor("binN", [NQB, HPC, NQB], BF, kind="ExternalInput").ap()
    out = nc.dram_tensor("out", [HPC * P, S], F32, kind="ExternalOutput").ap()

    with tile.TileContext(nc) as tc:
        with (
            tc.tile_pool(name="cp", bufs=1) as cp,          # persistent tensors
            tc.tile_pool(name="pp", bufs=1, space="PSUM") as pp,
            tc.tile_pool(name="dp", bufs=1, space="DRAM") as dp,
        ):
            QTr = cp.tile([P, HPC, S], BF, name="QTr")
            KTr = cp.tile([P, HPC, S], BF, name="KTr")
            V_sbs = [cp.tile([P, KT, P], BF, name=f"V_h{h}")
                     for h in range(HPC)]
            corrT_sb = cp.tile([NQB, HPC, P], BF, name="corrT_sb")

            # ---------------- input DMAs (queue assignment matters) --------
            qp = tc.alloc_tile_pool(name="qp", bufs=2)
            # scalar HWDGE: first K weight, then RoPE tables (needed at the
            # first K eviction ~7us in)
            wk_sbs, wv_sbs, wq_sbs = [], [], []
            wk0 = qp.tile([P, CH, P], BF, name="w_sb", tag="w_sb", bufs=6)
            nc.scalar.dma_start(wk0[:], wk[0])
            wk_sbs.append(wk0)
            cos_sb = qp.tile([P, S], F32, name="cos_sb", bufs=1)
            nc.scalar.dma_start(cos_sb[:], cosT[:])
            sin_sb = qp.tile([P, S], F32, name="sin_sb", bufs=1)
            nc.scalar.dma_start(sin_sb[:], sinT[:])
            # gpsimd SWDGE (slow ~9us spinup, fine for later-needed data)
            wk1 = qp.tile([P, CH, P], BF, name="w_sb", tag="w_sb", bufs=6)
            nc.gpsimd.dma_start(wk1[:], wk[1])
            wk_sbs.append(wk1)
            for h in range(HPC):
                wv_sb = qp.tile([P, CH, P], BF, name="w_sb", tag="w_sb", bufs=6)
                nc.gpsimd.dma_start(wv_sb[:], wv[h])
                wv_sbs.append(wv_sb)
            for h in range(HPC):
                wq_sb = qp.tile([P, CH, P], BF, name="w_sb", tag="w_sb", bufs=6)
                nc.gpsimd.dma_start(wq_sb[:], wq[h])
                wq_sbs.append(wq_sb)
            # hidden^T: 4 subtiles per chunk on the sync queue, tag-rotated
            # 2 chunks deep; the per-chunk K/K/V/V/Q/Q chains consume one
            # 2MB chunk per ~25us while sync delivers one per ~5.6us
            CSUB = 4
            hT_sbs = []
            for qcb in range(QC):
                subs = []
                for cs in range(CSUB):
                    hT_c = qp.tile([P, CH // CSUB, FB], BF,
                                   name=f"hT_c{cs}", tag=f"hT_c{cs}", bufs=2)
                    nc.sync.dma_start(
                        hT_c[:], hT[qcb, :, cs * (CH // CSUB):
                                     (cs + 1) * (CH // CSUB), :])
                    subs.append(hT_c)
                hT_sbs.append(subs)
            bin_sb = cp.tile([P, HPC, KT, NQB], BF, name="bin_sb")
            nc.gpsimd.dma_start(bin_sb[:], binT[:])
            binN_sb = cp.tile([NQB, HPC, NQB], BF, name="binN_sb")
            nc.gpsimd.dma_start(binN_sb[:], binN[:])
            wo_sb = cp.tile([P, CH, HPC * P], BF, name="wo_sb")
            nc.gpsimd.dma_start(wo_sb[:], wo[:])
            ones_sb = cp.tile([P, P], BF, name="ones_sb")
            nc.vector.memset(ones_sb[:], 1.0)
            c64_sb = cp.tile([NQB, P], BF, name="c64_sb")
            nc.vector.memset(c64_sb[:], float(BS))
            ident = cp.tile([P, P], BF, name="ident")
            make_identity(nc, ident[:])

            # ------------- phase 1: K, V, Q projections, chunk-major ------
            def emit_proj_chain(w_sb, qc):
                ps = pp.tile([P, FB], F32, name="ps_acc", tag="ps_acc",
                             bufs=2)
                for c in range(CH):
                    nc.tensor.matmul(
                        ps[:],
                        lhsT=w_sb[:, c, :],
                        rhs=hT_sbs[qc][c // (CH // CSUB)][
                            :, c % (CH // CSUB), :],
                        start=(c == 0),
                        stop=(c == CH - 1),
                    )
                return ps

            def emit_rope_evict(ps, dst, h, qc):
                qsl = slice(qc * FB, (qc + 1) * FB)
                tcos = qp.tile([P, FB], F32, name="tcos", tag="tcos")
                nc.vector.tensor_mul(
                    out=tcos[:], in0=ps[:], in1=cos_sb[:, qsl])
                tsin = qp.tile([P, FB], F32, name="tsin", tag="tsin")
                nc.vector.tensor_mul(
                    out=tsin[0:64, :], in0=ps[64:128, :],
                    in1=sin_sb[0:64, qsl])
                nc.vector.tensor_mul(
                    out=tsin[64:128, :], in0=ps[0:64, :],
                    in1=sin_sb[64:128, qsl])
                nc.vector.tensor_add(
                    out=dst[:, h, qsl], in0=tcos[:], in1=tsin[:])

            vT_sbs = {}
            for qc in range(QC):
                for h in range(HPC):
                    ps = emit_proj_chain(wk_sbs[h], qc)
                    emit_rope_evict(ps, KTr, h, qc)
                for h in range(HPC):
                    ps = emit_proj_chain(wv_sbs[h], qc)
                    vT_c = qp.tile([P, FB], BF, name="vT_c",
                                   tag="vT_c", bufs=8)
                    nc.scalar.copy(out=vT_c[:], in_=ps[:])
                    vT_sbs[(h, qc)] = vT_c
                    # natural-layout V via DMA transpose; on the sync queue
                    # (transposes burn issuing-engine time; sync is idle)
                    for j in range(QC):
                        kt = qc * QC + j
                        nc.sync.dma_start(
                            V_sbs[h][:, kt, :],
                            vT_c[:, j * P:(j + 1) * P],
                            transpose=True,
                        )
                for h in range(HPC):
                    ps = emit_proj_chain(wq_sbs[h], qc)
                    emit_rope_evict(ps, QTr, h, qc)
            # block-sums of V^T -> [d, kb] -> transpose -> corrT [kb, d]
            for h in range(HPC):
                bsum = qp.tile([P, NQB], BF, name="bsum", tag="bsum")
                with nc.allow_low_precision(
                        reason="block-sum correction term, 64-wide bf16 sum"):
                    for qc in range(QC):
                        nc.vector.tensor_reduce(
                            out=bsum[:, qc * QB_PER_FB:(qc + 1) * QB_PER_FB],
                            in_=vT_sbs[(h, qc)].rearrange(
                                "p (b e) -> p b e", e=BS),
                            axis=mybir.AxisListType.X,
                            op=mybir.AluOpType.add,
                        )
                ps_t = pp.tile([NQB, P], BF, name="ps_t", tag="ps_s", bufs=2)
                with nc.allow_low_precision(
                        reason="block-sum correction term, 64-wide bf16 sum"):
                    nc.tensor.transpose(ps_t[:], bsum[:], ident[:])
                nc.scalar.copy(out=corrT_sb[:, h, :], in_=ps_t[:])

            qp.release()

            # ------------- phase 2: attention + pipelined AG + o_proj -----
            cc_ins = {qc: dp.tile([HPC * P, FB], BF, name=f"cc_in{qc}")
                      for qc in range(QC - 1)}
            cc_outs = {qc: dp.tile([NCORES * HPC * P, FB], BF,
                                   name=f"cc_out{qc}", addr_space="Shared")
                       for qc in range(QC - 1)}
            # last chunk: per-head AGs so o_proj can overlap the final one
            cc_ins_s = {}
            cc_outs_s = {}
            for qc in (QC - 1,):
                for h in range(HPC):
                    cc_ins_s[(qc, h)] = dp.tile(
                        [P, FB], BF, name=f"cc_ins{qc}_{h}")
                    cc_outs_s[(qc, h)] = dp.tile(
                        [NCORES * P, FB], BF, name=f"cc_outs{qc}_{h}",
                        addr_space="Shared")

            def emit_oproj(qc):
                ag_sb = cp.tile([P, CH, FB], BF, name="ag_sb", tag="ag_sb",
                                bufs=2)
                nc.sync.dma_start(
                    ag_sb[:], cc_outs[qc].rearrange("(c p) s -> p c s", p=P))
                for strip in range(HPC):
                    ssl = slice(strip * P, (strip + 1) * P)
                    ps_w = pp.tile([P, FB], F32, name="ps_w", tag="ps_acc",
                                   bufs=2)
                    for c in range(CH):
                        nc.tensor.matmul(
                            ps_w[:],
                            lhsT=wo_sb[:, c, ssl],
                            rhs=ag_sb[:, c, :],
                            start=(c == 0),
                            stop=(c == CH - 1),
                        )
                    ot = cp.tile([P, FB], F32, name="ot", tag="ot", bufs=2)
                    nc.vector.tensor_copy(out=ot[:], in_=ps_w[:])
                    nc.sync.dma_start(
                        out[strip * P:(strip + 1) * P, qc * FB:(qc + 1) * FB],
                        ot[:],
                    )

            c_order = [c for c in range(CH) if c % HPC == 0] + \
                      [c for c in range(CH) if c % HPC != 0]

            def emit_oproj_split(qc):
                # per-head-gathered chunk: head 0's features consumed first
                # so work overlaps the second AG
                ags = []
                for hh in range(HPC):
                    ag_l = cp.tile([P, NCORES, FB], BF, name="ag_l",
                                   tag="ag_sb", bufs=2)
                    nc.sync.dma_start(
                        ag_l[:],
                        cc_outs_s[(qc, hh)].rearrange("(b p) s -> p b s", p=P))
                    ags.append(ag_l)
                for strip in range(HPC):
                    ssl = slice(strip * P, (strip + 1) * P)
                    ps_w = pp.tile([P, FB], F32, name="ps_w", tag="ps_acc",
                                   bufs=2)
                    for i, c in enumerate(c_order):
                        nc.tensor.matmul(
                            ps_w[:],
                            lhsT=wo_sb[:, c, ssl],
                            rhs=ags[c % HPC][:, c // HPC, :],
                            start=(i == 0),
                            stop=(i == CH - 1),
                        )
                    ot = cp.tile([P, FB], F32, name="ot", tag="ot", bufs=2)
                    nc.vector.tensor_copy(out=ot[:], in_=ps_w[:])
                    nc.sync.dma_start(
                        out[strip * P:(strip + 1) * P, qc * FB:(qc + 1) * FB],
                        ot[:],
                    )

            def emit_attention(h, qc):
                qsl = slice(qc * FB, (qc + 1) * FB)
                qbsl = slice(qc * QB_PER_FB, (qc + 1) * QB_PER_FB)
                pts = []
                dsums = []
                for ktp in range(NP_):
                    ps_s = pp.tile([P, KTB, FB], F32, name="ps_s",
                                   tag="ps_s", bufs=2)
                    for j in range(KTB):
                        kt = KTB * ktp + j
                        nc.tensor.matmul(
                            ps_s[:, j, :],
                            lhsT=KTr[:, h, kt * P:(kt + 1) * P],
                            rhs=QTr[:, h, qsl],
                            start=True, stop=True,
                        )
                    pt = cp.tile([P, KTB, FB], BF, name="probsT",
                                 tag="probsT", bufs=2 * NP_)
                    nc.scalar.activation(
                        out=pt[:], in_=ps_s[:],
                        func=mybir.ActivationFunctionType.Exp)
                    bin_ap = bin_sb[:, h, KTB * ktp:KTB * (ktp + 1), qbsl]
                    nc.vector.tensor_mul(
                        out=pt[:],
                        in0=pt[:],
                        in1=bin_ap[:, :, :, None].to_broadcast(
                            [P, KTB, QB_PER_FB, BS]),
                    )
                    pts.append(pt)
                    if ktp < NPAIR:
                        # pre-sum this key-tile pair for the denominator
                        dsum = cp.tile([P, FB], BF, name="dsum", tag="dsum",
                                       bufs=NPAIR + 2)
                        nc.vector.tensor_add(
                            out=dsum[:], in0=pt[:, 0, :], in1=pt[:, 1, :])
                        dsums.append(dsum)

                # PV + masked-block correction
                ps_o = pp.tile([P, FB], F32, name="ps_o", tag="ps_o", bufs=1)
                for kt in range(KT):
                    nc.tensor.matmul(
                        ps_o[:],
                        lhsT=V_sbs[h][:, kt, :],
                        rhs=pts[kt // KTB][:, kt % KTB, :],
                        start=(kt == 0), stop=False,
                    )
                binN_ap = binN_sb[:, h, qbsl]
                nc.tensor.matmul(
                    ps_o[:],
                    lhsT=corrT_sb[:, h, :],
                    rhs=binN_ap[:, :, None].to_broadcast(
                        [NQB, QB_PER_FB, BS]),
                    start=False, stop=True,
                )

                # denominator: 4 pre-summed pairs + 8 raw tiles + count corr
                ps_d = pp.tile([P, FB], F32, name="ps_d", tag="ps_d", bufs=1)
                first = True
                for i in range(NPAIR):
                    nc.tensor.matmul(
                        ps_d[:], lhsT=ones_sb[:], rhs=dsums[i][:],
                        start=first, stop=False)
                    first = False
                for kt in range(2 * NPAIR, KT):
                    nc.tensor.matmul(
                        ps_d[:],
                        lhsT=ones_sb[:],
                        rhs=pts[kt // KTB][:, kt % KTB, :],
                        start=False, stop=False,
                    )
                nc.tensor.matmul(
                    ps_d[:],
                    lhsT=c64_sb[:],
                    rhs=binN_ap[:, :, None].to_broadcast(
                        [NQB, QB_PER_FB, BS]),
                    start=False, stop=True,
                )
                rden = cp.tile([P, FB], F32, name="rden", tag="rden", bufs=2)
                nc.vector.reciprocal_approx_fast(out=rden[:], in_=ps_d[:])
                at_c = cp.tile([P, FB], BF, name="at_c", tag="at_c", bufs=4)
                nc.vector.tensor_mul(out=at_c[:], in0=ps_o[:], in1=rden[:])

                # deposit + trigger collective
                if qc < QC - 1:
                    nc.sync.dma_start(
                        cc_ins[qc][h * P:(h + 1) * P, :], at_c[:])
                    if h == HPC - 1:
                        nc.gpsimd.collective_compute(
                            "AllGather",
                            mybir.AluOpType.bypass,
                            replica_groups=[list(range(NCORES))],
                            ins=[cc_ins[qc].opt()],
                            outs=[cc_outs[qc].opt()],
                        )
                else:
                    nc.sync.dma_start(cc_ins_s[(qc, h)][:], at_c[:])
                    nc.gpsimd.collective_compute(
                        "AllGather",
                        mybir.AluOpType.bypass,
                        replica_groups=[list(range(NCORES))],
                        ins=[cc_ins_s[(qc, h)].opt()],
                        outs=[cc_outs_s[(qc, h)].opt()],
                    )

            # attention chunks with o_proj deferred so the first AllGather
            # (which absorbs cross-core launch skew) has plenty of slack
            for h in range(HPC):
                emit_attention(h, 0)
            for h in range(HPC):
                emit_attention(h, 1)
            for h in range(HPC):
                emit_attention(h, 2)
            emit_oproj(0)
            emit_oproj(1)
            for h in range(HPC):
                emit_attention(h, 3)
            emit_oproj(2)
            emit_oproj_split(QC - 1)

    nc.compile()
    return nc


def _host_prep(hidden_states, q_w, k_w, v_w, o_w, sparsity_pattern):
    hs = np.asarray(hidden_states, dtype=np.float32).reshape(S, HID)
    qw = np.asarray(q_w, dtype=np.float32)
    kw = np.asarray(k_w, dtype=np.float32)
    vw = np.asarray(v_w, dtype=np.float32)
    ow = np.asarray(o_w, dtype=np.float32)
    sp = np.asarray(sparsity_pattern, dtype=np.float32)

    bf = ml_dtypes.bfloat16

    # hidden^T -> [qcb, p, c, s'] (s-chunk-major so chunk DMAs are contiguous)
    hT = np.ascontiguousarray(
        hs.T.reshape(CH, P, QC, FB).transpose(2, 1, 0, 3)).astype(bf)

    # block mask with per-head top-k threshold
    kk = max(1, int(NH * NQB * NQB * RATIO / NH))
    flat = sp.reshape(NH, -1)
    th = np.partition(flat, -kk, axis=1)[:, -kk]
    bm = (sp > th[:, None, None]).astype(np.float32)  # [NH, 32 qb, 32 kb]

    # RoPE tables in [d, s] layout; sin pre-signed for rotate_half
    inv = 1.0 / (THETA ** (np.arange(0, HD, 2, dtype=np.float32) / HD))
    fr = np.arange(S, dtype=np.float32)[:, None] * inv[None, :]  # [S, 64]
    embT = np.ascontiguousarray(np.concatenate([fr, fr], axis=1).T)  # [128,S]
    cosT = np.cos(embT).astype(np.float32)
    sinT = np.sin(embT).astype(np.float32)
    sinT[:64] *= -1.0

    def w_per_head(w, h, scale=1.0):
        # [HID, 128] -> [p, c, d]
        return np.ascontiguousarray(
            (w[:, h * HD:(h + 1) * HD] * scale)
            .reshape(CH, P, HD).transpose(1, 0, 2))

    qscale = 1.0 / np.sqrt(HD)
    in_maps = []
    for r in range(NCORES):
        heads = [HPC * r + i for i in range(HPC)]
        wq_r = np.stack([w_per_head(qw, h, qscale) for h in heads]).astype(bf)
        wk_r = np.stack([w_per_head(kw, h) for h in heads]).astype(bf)
        wv_r = np.stack([w_per_head(vw, h) for h in heads]).astype(bf)
        wo_r = np.ascontiguousarray(
            ow[:, r * HPC * HD:(r + 1) * HPC * HD]
            .reshape(CH, P, HPC * HD).transpose(1, 0, 2)).astype(bf)
        # bm[h] is [q_block, k_block]; kernel layout wants keys on partitions
        mT = np.stack([
            np.repeat(bm[h].T, BS, axis=0).reshape(KT, P, NQB).transpose(1, 0, 2)
            for h in heads
        ], axis=1)  # [P, HPC, KT, NQB]
        # complement mask [kb, h, qb] for the masked-block corrections
        mN = np.stack([1.0 - bm[h].T for h in heads], axis=1)  # [32, HPC, 32]
        in_maps.append({
            "hT": hT,
            "wq": wq_r, "wk": wk_r, "wv": wv_r, "wo": wo_r,
            "cosT": cosT, "sinT": sinT,
            "binT": np.ascontiguousarray(mT).astype(bf),
            "binN": np.ascontiguousarray(mN).astype(bf),
        })
    return in_maps


def _run(inputs, trace=False, **kwargs):
    if "nc" not in _CACHE:
        _CACHE["nc"] = _build()
    nc = _CACHE["nc"]
    in_maps = _host_prep(**inputs)
    res = run_bass_kernel_spmd(
        nc, in_maps, core_ids=list(range(NCORES)), trace=trace, **kwargs)
    outT = np.empty((HID, S), dtype=np.float32)
    for r in range(NCORES):
        outT[r * HPC * P:(r + 1) * HPC * P] = \
            np.asarray(res.results[r]["out"], dtype=np.float32)
    full = np.ascontiguousarray(outT.T).reshape(B, S, HID)
    return full, res


def kernel(**inputs):
    full, _ = _run(inputs, trace=False)
    return full
